# revision 19
# baseline (speedup 1.0000x reference)
"""Chamfer + normal loss via spatially-pruned candidate scan on 8 trn2 cores.

Both clouds Morton-sorted on host (layout only; loss is a permutation-
invariant sum).  Padded to 22528 = 176 tiles/blocks of 128.  Each core owns
22 row-tiles per direction.  Per tile the device: (1) computes row-to-block-
centroid distances with a tiny matmul, (2) lower-bounds lb = dist - radius,
(3) takes the tile-min over rows via PE transposes + reduces, (4) selects the
top-32 blocks with max8/max_index/match_replace rounds, (5) gathers those 32
blocks' augmented columns via indirect DMA + DRAM-bounce relayout, (6) runs
the exact-ish K=24 bf16-split matmul on [128, 4096] candidates only, packs
bf16(d2)|local-idx words, min-reduces for winner+argmin, (7) batched decode
maps local->global indices.  Tail (gather + exact recompute + normal term +
AllReduce) as in the full-scan kernel."""

import os
import sys

for _p in ("/opt/trn_rl_repo", "/root/.axon_site/_ro/trn_rl_repo"):
    if os.path.isdir(_p) and _p not in sys.path:
        sys.path.append(_p)

import numpy as np
import ml_dtypes

CHAMFER_W = 1.0
NORMAL_W = 0.00016
EPS = 1e-6
SENTINEL = 100.0
B = 128          # block / tile size
L = 32           # candidate blocks per tile
CAND = L * B     # 4096
K = 24           # augmentation rows

# ---------------------------------------------------------------- host prep


def _bf(v):
    return v.astype(ml_dtypes.bfloat16).astype(np.float64)


def _split3(v):
    h = _bf(v)
    r = v - h
    l = _bf(r)
    l2 = _bf(r - l)
    return h, l, l2


def _aug_lhs(pts):
    n = pts.shape[0]
    out = np.zeros((K, n))
    x2 = (pts * pts).sum(1)
    for c in range(3):
        h, l, l2 = _split3(-2.0 * pts[:, c])
        base = 6 * c
        out[base + 0] = h
        out[base + 1] = h
        out[base + 2] = l
        out[base + 3] = l
        out[base + 4] = h
        out[base + 5] = l2
    h, l, l2 = _split3(x2)
    out[18] = 1.0
    out[19] = h
    out[20] = l
    out[21] = 1.0
    out[22] = l2
    out[23] = 1.0
    return out


def _aug_rhs(pts):
    m = pts.shape[0]
    out = np.zeros((K, m))
    y2 = (pts * pts).sum(1)
    for c in range(3):
        h, l, l2 = _split3(pts[:, c])
        base = 6 * c
        out[base + 0] = h
        out[base + 1] = l
        out[base + 2] = h
        out[base + 3] = l
        out[base + 4] = l2
        out[base + 5] = h
    h, l, l2 = _split3(y2)
    out[18] = h
    out[19] = 1.0
    out[20] = 1.0
    out[21] = l
    out[22] = 1.0
    out[23] = l2
    return out


def morton_order(pts, bits=7):
    q = pts - pts.min(0)
    q = (q / q.max() * ((1 << bits) - 1)).astype(np.uint32)
    code = np.zeros(len(pts), dtype=np.uint64)
    for i in range(bits):
        for d in range(3):
            code |= ((q[:, d].astype(np.uint64) >> i) & 1) << np.uint64(3 * i + d)
    return np.argsort(code, kind="stable")


def _loc_table(pts, normals, ntiles):
    n = pts.shape[0]
    total = ntiles * 128
    tab = np.zeros((total, 8), dtype=np.float64)
    tab[:n, 0:3] = pts
    tab[:n, 3] = (pts * pts).sum(1)
    tab[:n, 4:7] = normals
    tab[:n, 7] = np.sqrt((normals * normals).sum(1))
    tab = tab.reshape(ntiles, 128, 8).transpose(1, 0, 2).reshape(128, ntiles * 8)
    return tab.astype(np.float32)


def _side_tables(pts_s, normals_s, npad):
    """pts_s: [n,3] sorted float64. Returns dict of per-direction B-side
    (candidate side) tables: blocks [NB, K*128] bf16, cent aug [K, NB] bf16,
    radbc [128, NB] f32, tab [npad, 8] f32."""
    n = pts_s.shape[0]
    nb = npad // B
    pad = np.full((npad, 3), SENTINEL)
    pad[:n] = pts_s
    aug = _aug_rhs(pad).astype(ml_dtypes.bfloat16)          # [K, npad]
    blocks = np.ascontiguousarray(
        aug.reshape(K, nb, B).transpose(1, 0, 2).reshape(nb, K * B))
    cent = np.full((nb, 3), 1e3)
    rad = np.zeros(nb)
    for b in range(nb):
        lo, hi = b * B, min((b + 1) * B, n)
        if hi > lo:
            blk = pts_s[lo:hi]
            cent[b] = blk.mean(0)
            rad[b] = np.sqrt(((blk - cent[b]) ** 2).sum(1)).max()
    cent_aug = _aug_rhs(cent).astype(ml_dtypes.bfloat16)     # [K, nb]
    radbc = np.broadcast_to(rad.astype(np.float32), (128, nb)).copy()
    npts = np.zeros((npad, 3))
    npts[:n] = normals_s
    tab = np.zeros((npad, 8))
    tab[:, 0:3] = pad
    tab[:, 3] = (pad * pad).sum(1)
    tab[:, 4:7] = npts
    tab[:, 7] = np.sqrt((npts * npts).sum(1))
    return blocks, cent_aug, radbc, tab.astype(np.float32)


_CACHE = {}


def _build(params):
    import concourse.bacc as bacc
    import concourse.bass as bass
    import concourse.mybir as mybir
    from concourse import tile
    from concourse.masks import make_identity

    NT = params["ntiles"]          # row tiles per core per direction (22)
    NB = params["nblocks"]         # candidate blocks total (176)
    NCORES = params["ncores"]
    f32, bf16, i32, i16, u32 = (
        mybir.dt.float32, mybir.dt.bfloat16, mybir.dt.int32, mybir.dt.int16,
        mybir.dt.uint32,
    )
    f16 = mybir.dt.float16
    Copy = mybir.ActivationFunctionType.Copy
    Abs = mybir.ActivationFunctionType.Abs
    Sqrt = mybir.ActivationFunctionType.Sqrt
    A_ = mybir.AluOpType
    MIN, ADD, MULT, MAX, SUB = A_.min, A_.add, A_.mult, A_.max, A_.subtract
    IS_LE, IS_GT = A_.is_le, A_.is_gt
    X = mybir.AxisListType.X

    nc = bacc.Bacc("TRN2", target_bir_lowering=False, debug=False,
                   num_devices=NCORES)

    din = {}
    specs = [("mask", [128, NT], f32), ("w4", [4, 1], f32),
             ("ibuf", [128, CAND], i32), ("slotb", [128, L], f32),
             ("slotbh", [128, L], f32)]
    for d in ("x", "y"):
        specs += [
            (f"lhs{d}", [K, NT * 128], bf16),
            (f"blk{d}", [NB, K * B], bf16),
            (f"cent{d}", [K, NB], bf16),
            (f"radbc{d}", [128, NB], f32),
            (f"loc{d}", [128, NT * 8], f32),
            (f"tab{d}", [params["npad"], 8], f32),
        ]
    for name, shape, dt in specs:
        din[name] = nc.dram_tensor(name, shape, dt, kind="ExternalInput")
    d_out = nc.dram_tensor("out", [1, 1], f32, kind="ExternalOutput")
    d_sums = nc.dram_tensor("sums4", [4, 1], f32, kind="ExternalOutput")
    d_idx = {}
    for d in ("x", "y"):
        d_idx[d] = nc.dram_tensor(f"idx{d}", [128, NT], i32,
                                  kind="ExternalOutput")

    with tile.TileContext(nc) as tc:
        with tc.tile_pool(name="big", bufs=1) as bigpool, \
             tc.tile_pool(name="wk", bufs=2) as wk, \
             tc.tile_pool(name="wk3", bufs=3) as wk3, \
             tc.tile_pool(name="wkg", bufs=4) as wkg, \
             tc.tile_pool(name="fin", bufs=1) as fin, \
             tc.tile_pool(name="pscan", bufs=2, space="PSUM") as pscan, \
             tc.tile_pool(name="pcrs", bufs=2, space="PSUM") as pcrs, \
             tc.tile_pool(name="ptp", bufs=2, space="PSUM") as ptp, \
             tc.tile_pool(name="dram", bufs=3, space="DRAM") as dram:

            # ---- resident
            res = {}
            for name in ("mask", "slotb", "slotbh"):
                t_ = bigpool.tile(din[name].shape, din[name].dtype, tag=name,
                                  name=name)
                nc.sync.dma_start(out=t_[:], in_=din[name][:])
                res[name] = t_
            for d in ("x", "y"):
                for nm in (f"lhs{d}", f"cent{d}", f"radbc{d}", f"loc{d}"):
                    t_ = bigpool.tile(din[nm].shape, din[nm].dtype, tag=nm,
                                      name=nm)
                    nc.sync.dma_start(out=t_[:], in_=din[nm][:])
                    res[nm] = t_
            ident = bigpool.tile([128, 128], f32, tag="ident", name="ident")
            make_identity(nc, ident[:])
            sqb = bigpool.tile([128, 1], f32, tag="sqb", name="sqb")
            nc.vector.memset(sqb[:], 1e-4)
            ibufs = []
            for q in range(2):
                ib = bigpool.tile([128, CAND], f32, tag=f"ibuf{q}",
                                  name=f"ibuf{q}")
                nc.sync.dma_start(out=ib[:].bitcast(i32), in_=din["ibuf"][:])
                ibufs.append(ib)

            pkarr = {}
            btaball = {}
            for d in ("x", "y"):
                pkarr[d] = bigpool.tile([128, NT], f32, tag=f"pk{d}",
                                        name=f"pk{d}")
                btaball[d] = bigpool.tile([128, NT * L], i32, tag=f"bt{d}",
                                          name=f"bt{d}")

            # ---- main joint per-tile loop (x and y share top-L rounds)
            gidx_tiles = {}
            gath_tiles = {}
            idsall = {}
            for d in ("x", "y"):
                idsall[d] = dram.tile([NT, L], i32, tag=f"idsall{d}",
                                      name=f"idsall{d}")
            n1 = min(NB, 128)
            for t in range(NT):
                nv2 = wk.tile([2, max(NB, 8)], f32, tag="nv")
                tvj = wk.tile([128, 4], f32, tag="tv")
                for di, d in enumerate(("x", "y")):
                    lhs, cent, radbc = (res[f"lhs{d}"], res[f"cent{d}"],
                                        res[f"radbc{d}"])
                    lslice = lhs[:, t * 128:(t + 1) * 128]
                    # -- coarse: d2 to centroids
                    psc = pcrs.tile([128, NB], f32, tag="crs")
                    nc.tensor.matmul(out=psc[:], lhsT=lslice, rhs=cent[:],
                                     start=True, stop=True)
                    dc = wk.tile([128, NB], f32, tag="dc")
                    nc.scalar.activation(out=dc[:], in_=psc[:], func=Sqrt,
                                         bias=sqb[:])
                    nlb = wk.tile([128, NB], f32, tag="nlb")
                    nc.vector.tensor_tensor(out=nlb[:], in0=radbc[:],
                                            in1=dc[:], op=SUB)
                    # -- tile-max of -lb over rows (column di of tvj)
                    tp = ptp.tile([128, 256], f32, tag="tp")
                    nc.tensor.transpose(tp[0:n1, 0:128], nlb[:, 0:n1],
                                        ident[:])
                    nc.vector.tensor_reduce(out=tvj[0:n1, di:di + 1],
                                            in_=tp[0:n1, 0:128],
                                            axis=X, op=MAX)
                    if NB > 128:
                        n2 = NB - 128
                        nc.tensor.transpose(tp[0:n2, 128:256],
                                            nlb[:, 128:NB], ident[:])
                        nc.vector.tensor_reduce(out=tvj[0:n2, 2 + di:3 + di],
                                                in_=tp[0:n2, 128:256],
                                                axis=X, op=MAX)
                # -- joint transpose of both directions' tile-max vectors
                tpj = ptp.tile([128, 256], f32, tag="tp")
                nc.tensor.transpose(tpj[0:2, 0:n1], tvj[0:n1, 0:2],
                                    ident[0:n1, 0:n1])
                nc.scalar.activation(out=nv2[:, 0:n1], in_=tpj[0:2, 0:n1],
                                     func=Copy)
                if NB > 128:
                    n2 = NB - 128
                    nc.tensor.transpose(tpj[0:2, 128:128 + n2],
                                        tvj[0:n2, 2:4], ident[0:n2, 0:n2])
                    nc.scalar.activation(out=nv2[:, 128:NB],
                                         in_=tpj[0:2, 128:NB], func=Copy)
                # -- top-L blocks for both directions: 4 shared rounds
                m8 = wk.tile([2, 8], f32, tag="m8")
                i82 = wk.tile([2, L], u32, tag="i8")
                for r in range(L // 8):
                    nc.vector.max(out=m8[:], in_=nv2[:])
                    nc.vector.max_index(out=i82[:, r * 8:(r + 1) * 8],
                                        in_max=m8[:], in_values=nv2[:])
                    nc.vector.match_replace(out=nv2[:], in_to_replace=m8[:],
                                            in_values=nv2[:],
                                            imm_value=-1e30)
                for di, d in enumerate(("x", "y")):
                    lhs = res[f"lhs{d}"]
                    lslice = lhs[:, t * 128:(t + 1) * 128]
                    # -- ids -> dram; offsets read back partition-major
                    ids_sl = idsall[d][t:t + 1, :]
                    nc.sync.dma_start(out=ids_sl,
                                      in_=i82[di:di + 1, :].bitcast(i32))
                    offs = wkg.tile([L, 1], i32, tag="offs")
                    nc.sync.dma_start(
                        out=offs[:],
                        in_=ids_sl.rearrange("o (s u) -> (o s) u", u=1))
                    # -- gather slabs + bounce relayout
                    slab = wkg.tile([L, K * B], bf16, tag="slab")
                    nc.gpsimd.indirect_dma_start(
                        out=slab[:], out_offset=None,
                        in_=din[f"blk{d}"][:],
                        in_offset=bass.IndirectOffsetOnAxis(ap=offs[:], axis=0),
                    )
                    sd = dram.tile([L, K * B], bf16, tag="sd")
                    nc.sync.dma_start(out=sd[:], in_=slab[:])
                    rhs_t = wkg.tile([K, CAND], bf16, tag="rhs")
                    nc.sync.dma_start(
                        out=rhs_t[:].rearrange("k (s c) -> k s c", c=B),
                        in_=sd[:].rearrange("s (k c) -> k s c", c=B),
                    )
                    # -- candidate scan: 4 groups of 1024
                    ib = ibufs[di]
                    ibh = ib[:].bitcast(f16)
                    acc = wk.tile([128, 4], f32, tag="acc")
                    for g in range(4):
                        ps = pscan.tile([128, 1024], f32, tag="scan")
                        for m in range(2):
                            c0 = g * 1024 + m * 512
                            nc.tensor.matmul(
                                out=ps[:, m * 512:(m + 1) * 512],
                                lhsT=lslice,
                                rhs=rhs_t[:, c0:c0 + 512],
                                start=True, stop=True,
                            )
                        nc.scalar.activation(
                            out=ibh[:, 2 * g * 1024 + 1: 2 * (g + 1) * 1024: 2],
                            in_=ps[:], func=Copy,
                        )
                        nc.vector.tensor_reduce(
                            out=acc[:, g:g + 1],
                            in_=ib[:, g * 1024:(g + 1) * 1024], axis=X, op=MIN,
                        )
                    nc.vector.tensor_reduce(
                        out=pkarr[d][:, t:t + 1], in_=acc[:], axis=X, op=MIN,
                    )

            for d in ("x", "y"):
                # ---- batched decode for direction d
                # gidx = sum_s oh_s * (btab_s*128 - slotbase_s) + idx_local
                # oh_s = (idx >= s*128) * (idx < (s+1)*128)
                src = idsall[d][:]
                nc.sync.dma_start(
                    out=btaball[d][:],
                    in_=bass.AP(src.tensor, src.offset,
                                [[0, 128], [1, NT * L]]))
                idxloc = wk.tile([128, NT], i32, tag="idxloc")
                nc.vector.tensor_copy(
                    out=idxloc[:], in_=pkarr[d][:].bitcast(i16)[:, 0::2])
                idxf = wk.tile([128, NT], f32, tag="idxf")
                nc.vector.tensor_copy(out=idxf[:], in_=idxloc[:])
                btf = wk.tile([128, NT * L], f32, tag="btf")
                nc.vector.tensor_copy(out=btf[:], in_=btaball[d][:])
                nc.vector.tensor_scalar(out=btf[:], in0=btf[:], scalar1=128.0,
                                        scalar2=None, op0=MULT)
                slot3 = res["slotb"][:].rearrange(
                    "p (o l) -> p o l", o=1).broadcast_to([128, NT, L])
                sloth3 = res["slotbh"][:].rearrange(
                    "p (o l) -> p o l", o=1).broadcast_to([128, NT, L])
                idx3 = idxf[:].rearrange(
                    "p (t o) -> p t o", o=1).broadcast_to([128, NT, L])
                btf3 = btf[:].rearrange("p (t l) -> p t l", l=L)
                nc.vector.tensor_tensor(out=btf3, in0=btf3, in1=slot3, op=SUB)
                c1 = wk.tile([128, NT * L], f32, tag="c1")
                c13 = c1[:].rearrange("p (t l) -> p t l", l=L)
                nc.vector.tensor_tensor(out=c13, in0=idx3, in1=slot3,
                                        op=A_.is_ge)
                c2 = wk.tile([128, NT * L], f32, tag="c2")
                c23 = c2[:].rearrange("p (t l) -> p t l", l=L)
                nc.vector.tensor_tensor(out=c23, in0=idx3, in1=sloth3,
                                        op=A_.is_lt)
                nc.vector.tensor_tensor(out=c1[:], in0=c1[:], in1=c2[:],
                                        op=MULT)
                nc.vector.tensor_tensor(out=btf[:], in0=btf[:], in1=c1[:],
                                        op=MULT)
                gidxf = wk.tile([128, NT], f32, tag="gidxf")
                nc.vector.tensor_reduce(
                    out=gidxf[:], in_=btf[:].rearrange("p (t l) -> p t l", l=L),
                    axis=X, op=ADD)
                nc.vector.tensor_tensor(out=gidxf[:], in0=gidxf[:],
                                        in1=idxf[:], op=ADD)
                gidx = fin.tile([128, NT], i32, tag=f"gidx{d}",
                                name=f"gidx{d}")
                nc.vector.tensor_copy(out=gidx[:], in_=gidxf[:])
                gidx_tiles[d] = gidx
                nc.sync.dma_start(out=d_idx[d][:], in_=gidx[:])
                # issue tail gathers now so they overlap the other
                # direction's decode work
                gath = fin.tile([128, NT * 8], f32, tag=f"gath{d}",
                                name=f"gath{d}")
                for t in range(NT):
                    nc.gpsimd.indirect_dma_start(
                        out=gath[:, t * 8:(t + 1) * 8], out_offset=None,
                        in_=din[f"tab{d}"][:],
                        in_offset=bass.IndirectOffsetOnAxis(
                            ap=gidx[:, t:t + 1], axis=0),
                    )
                gath_tiles[d] = gath

            # ---- tail: exact recompute + normal terms
            sums = fin.tile([128, 4], f32)
            maskt = res["mask"]
            for j, d in enumerate(("x", "y")):
                gath = gath_tiles[d]
                Lc = res[f"loc{d}"][:]
                L3 = Lc.rearrange("p (t k) -> p t k", k=8)
                G3 = gath[:].rearrange("p (t k) -> p t k", k=8)
                prod = wk3.tile([128, NT * 3], f32, tag="prod")
                nc.vector.tensor_tensor(
                    out=prod[:].rearrange("p (t k) -> p t k", k=3),
                    in0=L3[:, :, 0:3], in1=G3[:, :, 0:3], op=MULT)
                dot = wk3.tile([128, NT], f32, tag="sm")
                nc.vector.tensor_reduce(
                    out=dot[:], in_=prod[:].rearrange("p (t k) -> p t k", k=3),
                    axis=X, op=ADD)
                cham = wk3.tile([128, NT], f32, tag="sm")
                nc.vector.tensor_tensor(
                    out=cham[:], in0=L3[:, :, 3], in1=G3[:, :, 3], op=ADD)
                dotm2 = wk3.tile([128, NT], f32, tag="sm")
                nc.vector.tensor_scalar(
                    out=dotm2[:], in0=dot[:], scalar1=-2.0, scalar2=None,
                    op0=MULT)
                nc.vector.tensor_tensor(out=cham[:], in0=cham[:], in1=dotm2[:],
                                        op=ADD)
                nc.vector.tensor_tensor(out=cham[:], in0=cham[:],
                                        in1=maskt[:], op=MULT)
                nc.vector.tensor_reduce(out=sums[:, j:j + 1], in_=cham[:],
                                        axis=X, op=ADD)
                nprod = wk3.tile([128, NT * 3], f32, tag="prod")
                nc.vector.tensor_tensor(
                    out=nprod[:].rearrange("p (t k) -> p t k", k=3),
                    in0=L3[:, :, 4:7], in1=G3[:, :, 4:7], op=MULT)
                ndot = wk3.tile([128, NT], f32, tag="sm")
                nc.vector.tensor_reduce(
                    out=ndot[:], in_=nprod[:].rearrange("p (t k) -> p t k", k=3),
                    axis=X, op=ADD)
                den = wk3.tile([128, NT], f32, tag="sm")
                nc.vector.tensor_tensor(out=den[:], in0=L3[:, :, 7],
                                        in1=G3[:, :, 7], op=MULT)
                nc.vector.tensor_scalar(out=den[:], in0=den[:], scalar1=EPS,
                                        scalar2=None, op0=MAX)
                rec = wk3.tile([128, NT], f32, tag="sm")
                nc.vector.reciprocal(out=rec[:], in_=den[:])
                cos = wk3.tile([128, NT], f32, tag="sm")
                nc.vector.tensor_tensor(out=cos[:], in0=ndot[:], in1=rec[:],
                                        op=MULT)
                acos = wk3.tile([128, NT], f32, tag="sm")
                nc.scalar.activation(out=acos[:], in_=cos[:], func=Abs)
                nterm = wk3.tile([128, NT], f32, tag="sm")
                nc.scalar.activation(out=nterm[:], in_=acos[:], func=Copy,
                                     scale=-1.0, bias=1.0)
                nc.vector.tensor_tensor(out=nterm[:], in0=nterm[:],
                                        in1=maskt[:], op=MULT)
                nc.vector.tensor_reduce(out=sums[:, 2 + j:3 + j], in_=nterm[:],
                                        axis=X, op=ADD)

            # ---- partition reduce + weights + allreduce
            ones = fin.tile([128, 1], f32)
            nc.vector.memset(ones[:], 1.0)
            ps4full = pscan.tile([128, 1024], f32, tag="scan", name="ps4full")
            ps4 = ps4full[:4, :1]
            nc.tensor.matmul(out=ps4, lhsT=sums[:], rhs=ones[:],
                             start=True, stop=True)
            sb4 = fin.tile([4, 1], f32)
            nc.scalar.activation(out=sb4[:], in_=ps4, func=Copy)
            nc.sync.dma_start(out=d_sums[:], in_=sb4[:])
            w4 = fin.tile([4, 1], f32)
            nc.sync.dma_start(out=w4[:], in_=din["w4"][:])
            ps1full = pscan.tile([128, 1024], f32, tag="scan", name="ps1full")
            ps1 = ps1full[:1, :1]
            nc.tensor.matmul(out=ps1, lhsT=w4[:], rhs=sb4[:],
                             start=True, stop=True)
            sres = fin.tile([1, 1], f32)
            nc.scalar.activation(out=sres[:], in_=ps1, func=Copy)

            cc_in = dram.tile([1, 1], f32)
            cc_out = dram.tile([1, 1], f32)
            nc.sync.dma_start(out=cc_in[:], in_=sres[:])
            nc.gpsimd.collective_compute(
                "AllReduce", ADD,
                replica_groups=[list(range(NCORES))],
                ins=[cc_in.opt()], outs=[cc_out.opt()],
            )
            resid = fin.tile([1, 1], f32)
            nc.sync.dma_start(out=resid[:], in_=cc_out[:])
            nc.sync.dma_start(out=d_out[:], in_=resid[:])

    nc.compile()
    return nc


def _prepare_inputs(points_pred, normals_pred, points_gt, normals_gt, params):
    NT, NCORES, NPAD = params["ntiles"], params["ncores"], params["npad"]
    n, m = params["n"], params["m"]
    pp64 = points_pred.astype(np.float64)
    gg64 = points_gt.astype(np.float64)
    po = morton_order(pp64)
    go = morton_order(gg64)
    pp_s, pn_s = pp64[po], normals_pred.astype(np.float64)[po]
    gg_s, gn_s = gg64[go], normals_gt.astype(np.float64)[go]

    # B-side (candidate) tables per direction
    blkx, centx, radbcx, tabx = _side_tables(gg_s, gn_s, NPAD)   # x scans gt
    blky, centy, radbcy, taby = _side_tables(pp_s, pn_s, NPAD)   # y scans pred

    ibuf = np.broadcast_to(
        (np.int64(0x7F7F0000) | np.arange(CAND, dtype=np.int64))
        .astype(np.int32), (128, CAND)).copy()
    slotb = np.broadcast_to(
        (np.arange(L, dtype=np.float32) * 128), (128, L)).copy()
    slotbh = slotb + 128.0
    w4 = np.array(
        [[CHAMFER_W / n], [CHAMFER_W / m],
         [NORMAL_W / n], [NORMAL_W / m]], dtype=np.float32)

    local = NPAD // NCORES
    ppad = np.full((NPAD, 3), SENTINEL)
    ppad[:n] = pp_s
    gpad = np.full((NPAD, 3), SENTINEL)
    gpad[:m] = gg_s
    pnpad = np.zeros((NPAD, 3))
    pnpad[:n] = pn_s
    gnpad = np.zeros((NPAD, 3))
    gnpad[:m] = gn_s

    in_maps = []
    for i in range(NCORES):
        sel = slice(i * local, (i + 1) * local)
        lhsx = _aug_lhs(ppad[sel]).astype(ml_dtypes.bfloat16)
        lhsy = _aug_lhs(gpad[sel]).astype(ml_dtypes.bfloat16)
        locx = _loc_table(ppad[sel], pnpad[sel], NT)
        locy = _loc_table(gpad[sel], gnpad[sel], NT)
        gi = np.arange(i * local, (i + 1) * local)
        maskv = (gi < n).astype(np.float32)  # n == m
        mask = maskv.reshape(NT, 128).T.copy()
        in_maps.append({
            "lhsx": lhsx, "lhsy": lhsy,
            "blkx": blkx, "blky": blky,
            "centx": centx, "centy": centy,
            "radbcx": radbcx, "radbcy": radbcy,
            "locx": locx, "locy": locy,
            "tabx": tabx, "taby": taby,
            "mask": mask, "w4": w4, "ibuf": ibuf, "slotb": slotb,
            "slotbh": slotbh,
        })
    return in_maps


def _params_for(n, m, ncores=8):
    npad = ((max(n, m) + B * ncores - 1) // (B * ncores)) * (B * ncores)
    return {
        "n": n, "m": m, "ncores": ncores, "npad": npad,
        "ntiles": npad // B // ncores,
        "nblocks": npad // B,
    }


def run(points_pred, normals_pred, points_gt, normals_gt, ncores=8, **runkw):
    b, n, _ = points_pred.shape
    m = points_gt.shape[1]
    assert b == 1
    params = _params_for(n, m, ncores)
    key = (n, m, ncores)
    if key not in _CACHE:
        _CACHE[key] = _build(params)
    nc = _CACHE[key]
    in_maps = _prepare_inputs(
        points_pred[0], normals_pred[0], points_gt[0], normals_gt[0], params)
    from concourse.bass_utils import run_bass_kernel_spmd
    return run_bass_kernel_spmd(nc, in_maps, list(range(ncores)), **runkw)


def kernel(points_pred, normals_pred, points_gt, normals_gt):
    r = run(points_pred, normals_pred, points_gt, normals_gt)
    return np.float32(r.results[0]["out"][0, 0])


# revision 25
# speedup vs baseline: 1.1117x; 1.1117x over previous
"""Chamfer + normal loss via spatially-pruned candidate scan on 8 trn2 cores.

Both clouds Morton-sorted on host (layout only; loss is a permutation-
invariant sum).  Padded to 22528 = 176 tiles/blocks of 128.  Each core owns
22 row-tiles per direction.  Per tile the device: (1) computes row-to-block-
centroid distances with a tiny matmul, (2) lower-bounds lb = dist - radius,
(3) takes the tile-min over rows via PE transposes + reduces, (4) selects the
top-32 blocks with max8/max_index/match_replace rounds, (5) gathers those 32
blocks' augmented columns via indirect DMA + DRAM-bounce relayout, (6) runs
the exact-ish K=24 bf16-split matmul on [128, 4096] candidates only, packs
bf16(d2)|local-idx words, min-reduces for winner+argmin, (7) batched decode
maps local->global indices.  Tail (gather + exact recompute + normal term +
AllReduce) as in the full-scan kernel."""

import os
import sys

for _p in ("/opt/trn_rl_repo", "/root/.axon_site/_ro/trn_rl_repo"):
    if os.path.isdir(_p) and _p not in sys.path:
        sys.path.append(_p)

import numpy as np
import ml_dtypes

CHAMFER_W = 1.0
NORMAL_W = 0.00016
EPS = 1e-6
SENTINEL = 100.0
B = 128          # block / tile size
L = 32           # candidate blocks per tile
CAND = L * B     # 4096
K = 24           # augmentation rows

# ---------------------------------------------------------------- host prep


def _bf(v):
    return v.astype(ml_dtypes.bfloat16).astype(np.float64)


def _split3(v):
    h = _bf(v)
    r = v - h
    l = _bf(r)
    l2 = _bf(r - l)
    return h, l, l2


def _aug_lhs(pts):
    n = pts.shape[0]
    out = np.zeros((K, n))
    x2 = (pts * pts).sum(1)
    for c in range(3):
        h, l, l2 = _split3(-2.0 * pts[:, c])
        base = 6 * c
        out[base + 0] = h
        out[base + 1] = h
        out[base + 2] = l
        out[base + 3] = l
        out[base + 4] = h
        out[base + 5] = l2
    h, l, l2 = _split3(x2)
    out[18] = 1.0
    out[19] = h
    out[20] = l
    out[21] = 1.0
    out[22] = l2
    out[23] = 1.0
    return out


def _aug_rhs(pts):
    m = pts.shape[0]
    out = np.zeros((K, m))
    y2 = (pts * pts).sum(1)
    for c in range(3):
        h, l, l2 = _split3(pts[:, c])
        base = 6 * c
        out[base + 0] = h
        out[base + 1] = l
        out[base + 2] = h
        out[base + 3] = l
        out[base + 4] = l2
        out[base + 5] = h
    h, l, l2 = _split3(y2)
    out[18] = h
    out[19] = 1.0
    out[20] = 1.0
    out[21] = l
    out[22] = 1.0
    out[23] = l2
    return out


def morton_order(pts, bits=7):
    q = pts - pts.min(0)
    q = (q / q.max() * ((1 << bits) - 1)).astype(np.uint32)
    code = np.zeros(len(pts), dtype=np.uint64)
    for i in range(bits):
        for d in range(3):
            code |= ((q[:, d].astype(np.uint64) >> i) & 1) << np.uint64(3 * i + d)
    return np.argsort(code, kind="stable")


def _loc_table(pts, normals, ntiles):
    n = pts.shape[0]
    total = ntiles * 128
    tab = np.zeros((total, 8), dtype=np.float64)
    tab[:n, 0:3] = pts
    tab[:n, 3] = (pts * pts).sum(1)
    tab[:n, 4:7] = normals
    tab[:n, 7] = np.sqrt((normals * normals).sum(1))
    tab = tab.reshape(ntiles, 128, 8).transpose(1, 0, 2).reshape(128, ntiles * 8)
    return tab.astype(np.float32)


def _side_tables(pts_s, normals_s, npad):
    """pts_s: [n,3] sorted float64. Returns dict of per-direction B-side
    (candidate side) tables: blocks [NB, K*128] bf16, cent aug [K, NB] bf16,
    radbc [128, NB] f32, tab [npad, 8] f32."""
    n = pts_s.shape[0]
    nb = npad // B
    pad = np.full((npad, 3), SENTINEL)
    pad[:n] = pts_s
    aug = _aug_rhs(pad).astype(ml_dtypes.bfloat16)          # [K, npad]
    blocks = np.ascontiguousarray(
        aug.reshape(K, nb, B).transpose(1, 0, 2).reshape(nb, K * B))
    cent = np.full((nb, 3), 1e3)
    rad = np.zeros(nb)
    for b in range(nb):
        lo, hi = b * B, min((b + 1) * B, n)
        if hi > lo:
            blk = pts_s[lo:hi]
            cent[b] = blk.mean(0)
            rad[b] = np.sqrt(((blk - cent[b]) ** 2).sum(1)).max()
    cent_aug = _aug_rhs(cent).astype(ml_dtypes.bfloat16)     # [K, nb]
    radbc = np.broadcast_to(rad.astype(np.float32), (128, nb)).copy()
    npts = np.zeros((npad, 3))
    npts[:n] = normals_s
    tab = np.zeros((npad, 8))
    tab[:, 0:3] = pad
    tab[:, 3] = (pad * pad).sum(1)
    tab[:, 4:7] = npts
    tab[:, 7] = np.sqrt((npts * npts).sum(1))
    return blocks, cent_aug, radbc, tab.astype(np.float32)


_CACHE = {}


def _build(params):
    import concourse.bacc as bacc
    import concourse.bass as bass
    import concourse.mybir as mybir
    from concourse import tile
    from concourse.masks import make_identity

    NT = params["ntiles"]          # row tiles per core per direction (22)
    NB = params["nblocks"]         # candidate blocks total (176)
    NCORES = params["ncores"]
    f32, bf16, i32, i16, u32 = (
        mybir.dt.float32, mybir.dt.bfloat16, mybir.dt.int32, mybir.dt.int16,
        mybir.dt.uint32,
    )
    f16 = mybir.dt.float16
    Copy = mybir.ActivationFunctionType.Copy
    Abs = mybir.ActivationFunctionType.Abs
    Sqrt = mybir.ActivationFunctionType.Sqrt
    A_ = mybir.AluOpType
    MIN, ADD, MULT, MAX, SUB = A_.min, A_.add, A_.mult, A_.max, A_.subtract
    IS_LE, IS_GT = A_.is_le, A_.is_gt
    X = mybir.AxisListType.X

    nc = bacc.Bacc("TRN2", target_bir_lowering=False, debug=False,
                   num_devices=NCORES)

    din = {}
    specs = [("mask", [128, NT], f32), ("w4", [4, 1], f32),
             ("ibuf", [128, CAND], i32), ("slotb", [128, L], f32),
             ("slotbh", [128, L], f32)]
    for d in ("x", "y"):
        specs += [
            (f"lhs{d}", [K, NT * 128], bf16),
            (f"blk{d}", [NB, K * B], bf16),
            (f"cent{d}", [K, NB], bf16),
            (f"radbc{d}", [128, NB], f32),
            (f"loc{d}", [128, NT * 8], f32),
            (f"tab{d}", [params["npad"], 8], f32),
        ]
    for name, shape, dt in specs:
        din[name] = nc.dram_tensor(name, shape, dt, kind="ExternalInput")
    d_out = nc.dram_tensor("out", [1, 1], f32, kind="ExternalOutput")
    d_sums = nc.dram_tensor("sums4", [4, 1], f32, kind="ExternalOutput")
    d_idx = {}
    for d in ("x", "y"):
        d_idx[d] = nc.dram_tensor(f"idx{d}", [128, NT], i32,
                                  kind="ExternalOutput")

    with tile.TileContext(nc) as tc:
        with tc.tile_pool(name="big", bufs=1) as bigpool, \
             tc.tile_pool(name="wk", bufs=2) as wk, \
             tc.tile_pool(name="wk3", bufs=3) as wk3, \
             tc.tile_pool(name="wkg", bufs=4) as wkg, \
             tc.tile_pool(name="fin", bufs=1) as fin, \
             tc.tile_pool(name="pscan", bufs=4, space="PSUM") as pscan, \
             tc.tile_pool(name="pcrs", bufs=2, space="PSUM") as pcrs, \
             tc.tile_pool(name="ptp", bufs=2, space="PSUM") as ptp, \
             tc.tile_pool(name="dram", bufs=4, space="DRAM") as dram:

            # ---- resident
            res = {}
            for name in ("mask", "slotb", "slotbh"):
                t_ = bigpool.tile(din[name].shape, din[name].dtype, tag=name,
                                  name=name)
                nc.sync.dma_start(out=t_[:], in_=din[name][:])
                res[name] = t_
            for d in ("x", "y"):
                for nm in (f"lhs{d}", f"cent{d}", f"radbc{d}", f"loc{d}"):
                    t_ = bigpool.tile(din[nm].shape, din[nm].dtype, tag=nm,
                                      name=nm)
                    nc.sync.dma_start(out=t_[:], in_=din[nm][:])
                    res[nm] = t_
            ident = bigpool.tile([128, 128], f32, tag="ident", name="ident")
            make_identity(nc, ident[:])
            sqb = bigpool.tile([128, 1], f32, tag="sqb", name="sqb")
            nc.vector.memset(sqb[:], 1e-4)
            ibufs = []
            for q in range(4):
                ib = bigpool.tile([128, CAND], f32, tag=f"ibuf{q}",
                                  name=f"ibuf{q}")
                if q == 0:
                    nc.sync.dma_start(out=ib[:].bitcast(i32),
                                      in_=din["ibuf"][:])
                else:
                    nc.vector.tensor_copy(out=ib[:].bitcast(i32),
                                          in_=ibufs[0][:].bitcast(i32))
                ibufs.append(ib)

            pkarr = {}
            btaball = {}
            for d in ("x", "y"):
                pkarr[d] = bigpool.tile([128, NT], f32, tag=f"pk{d}",
                                        name=f"pk{d}")
                btaball[d] = bigpool.tile([128, NT * L], i32, tag=f"bt{d}",
                                          name=f"bt{d}")

            # ---- main joint per-tile loop (x and y share top-L rounds)
            gidx_tiles = {}
            gath_tiles = {}
            idsall = {}
            for d in ("x", "y"):
                idsall[d] = dram.tile([NT, L], i32, tag=f"idsall{d}",
                                      name=f"idsall{d}")
            n1 = min(NB, 128)
            for t in range(NT):
                nv2 = wk.tile([2, max(NB, 8)], f32, tag="nv")
                tvj = wk.tile([128, 4], f32, tag="tv")
                for di, d in enumerate(("x", "y")):
                    lhs, cent, radbc = (res[f"lhs{d}"], res[f"cent{d}"],
                                        res[f"radbc{d}"])
                    lslice = lhs[:, t * 128:(t + 1) * 128]
                    # -- coarse: d2 to centroids
                    psc = pcrs.tile([128, NB], f32, tag="crs")
                    nc.tensor.matmul(out=psc[:], lhsT=lslice, rhs=cent[:],
                                     start=True, stop=True)
                    dc = wk.tile([128, NB], f32, tag="dc")
                    nc.scalar.activation(out=dc[:], in_=psc[:], func=Sqrt,
                                         bias=sqb[:])
                    nlb = wk.tile([128, NB], f32, tag="nlb")
                    nc.vector.tensor_tensor(out=nlb[:], in0=radbc[:],
                                            in1=dc[:], op=SUB)
                    # -- tile-max of -lb over rows (column di of tvj)
                    tp = ptp.tile([128, 256], f32, tag="tp")
                    nc.tensor.transpose(tp[0:n1, 0:128], nlb[:, 0:n1],
                                        ident[:])
                    nc.vector.tensor_reduce(out=tvj[0:n1, di:di + 1],
                                            in_=tp[0:n1, 0:128],
                                            axis=X, op=MAX)
                    if NB > 128:
                        n2 = NB - 128
                        nc.tensor.transpose(tp[0:n2, 128:256],
                                            nlb[:, 128:NB], ident[:])
                        nc.vector.tensor_reduce(out=tvj[0:n2, 2 + di:3 + di],
                                                in_=tp[0:n2, 128:256],
                                                axis=X, op=MAX)
                # -- joint transpose of both directions' tile-max vectors
                tpj = ptp.tile([128, 256], f32, tag="tp")
                nc.tensor.transpose(tpj[0:2, 0:n1], tvj[0:n1, 0:2],
                                    ident[0:n1, 0:n1])
                nc.scalar.activation(out=nv2[:, 0:n1], in_=tpj[0:2, 0:n1],
                                     func=Copy)
                if NB > 128:
                    n2 = NB - 128
                    nc.tensor.transpose(tpj[0:2, 128:128 + n2],
                                        tvj[0:n2, 2:4], ident[0:n2, 0:n2])
                    nc.scalar.activation(out=nv2[:, 128:NB],
                                         in_=tpj[0:2, 128:NB], func=Copy)
                # -- top-L blocks for both directions: 4 shared rounds
                m8 = wkg.tile([2, 8], f32, tag="m8")
                i82 = wkg.tile([2, L], u32, tag="i8")
                for r in range(L // 8):
                    nc.vector.max(out=m8[:], in_=nv2[:])
                    nc.vector.max_index(out=i82[:, r * 8:(r + 1) * 8],
                                        in_max=m8[:], in_values=nv2[:])
                    nc.vector.match_replace(out=nv2[:], in_to_replace=m8[:],
                                            in_values=nv2[:],
                                            imm_value=-1e30)
                for di, d in enumerate(("x", "y")):
                    lhs = res[f"lhs{d}"]
                    lslice = lhs[:, t * 128:(t + 1) * 128]
                    # -- ids -> dram; offsets read back partition-major
                    ids_sl = idsall[d][t:t + 1, :]
                    nc.sync.dma_start(out=ids_sl,
                                      in_=i82[di:di + 1, :].bitcast(i32))
                    offs = wkg.tile([L, 1], i32, tag="offs")
                    nc.sync.dma_start(
                        out=offs[:],
                        in_=ids_sl.rearrange("o (s u) -> (o s) u", u=1))
                    # -- gather slabs + bounce relayout
                    slab = wkg.tile([L, K * B], bf16, tag="slab")
                    nc.gpsimd.indirect_dma_start(
                        out=slab[:], out_offset=None,
                        in_=din[f"blk{d}"][:],
                        in_offset=bass.IndirectOffsetOnAxis(ap=offs[:], axis=0),
                    )
                    sd = dram.tile([L, K * B], bf16, tag="sd")
                    nc.sync.dma_start(out=sd[:], in_=slab[:])
                    rhs_t = wkg.tile([K, CAND], bf16, tag="rhs")
                    nc.sync.dma_start(
                        out=rhs_t[:].rearrange("k (s c) -> k s c", c=B),
                        in_=sd[:].rearrange("s (k c) -> k s c", c=B),
                    )
                    # -- candidate scan: 4 groups of 1024
                    ib = ibufs[di * 2 + t % 2]
                    ibh = ib[:].bitcast(f16)
                    acc = wk.tile([128, 4], f32, tag="acc")
                    for g in range(4):
                        ps = pscan.tile([128, 1024], f32, tag="scan")
                        for m in range(2):
                            c0 = g * 1024 + m * 512
                            nc.tensor.matmul(
                                out=ps[:, m * 512:(m + 1) * 512],
                                lhsT=lslice,
                                rhs=rhs_t[:, c0:c0 + 512],
                                start=True, stop=True,
                            )
                        nc.scalar.activation(
                            out=ibh[:, 2 * g * 1024 + 1: 2 * (g + 1) * 1024: 2],
                            in_=ps[:], func=Copy,
                        )
                        nc.vector.tensor_reduce(
                            out=acc[:, g:g + 1],
                            in_=ib[:, g * 1024:(g + 1) * 1024], axis=X, op=MIN,
                        )
                    nc.vector.tensor_reduce(
                        out=pkarr[d][:, t:t + 1], in_=acc[:], axis=X, op=MIN,
                    )

            for d in ("x", "y"):
                # ---- batched decode for direction d
                # gidx = sum_s oh_s * (btab_s*128 - slotbase_s) + idx_local
                # oh_s = (idx >= s*128) * (idx < (s+1)*128)
                src = idsall[d][:]
                nc.sync.dma_start(
                    out=btaball[d][:],
                    in_=bass.AP(src.tensor, src.offset,
                                [[0, 128], [1, NT * L]]))
                idxloc = wk.tile([128, NT], i32, tag="idxloc")
                nc.vector.tensor_copy(
                    out=idxloc[:], in_=pkarr[d][:].bitcast(i16)[:, 0::2])
                idxf = wk.tile([128, NT], f32, tag="idxf")
                nc.vector.tensor_copy(out=idxf[:], in_=idxloc[:])
                btf = wk.tile([128, NT * L], f32, tag="btf")
                nc.vector.tensor_copy(out=btf[:], in_=btaball[d][:])
                nc.vector.tensor_scalar(out=btf[:], in0=btf[:], scalar1=128.0,
                                        scalar2=None, op0=MULT)
                slot3 = res["slotb"][:].rearrange(
                    "p (o l) -> p o l", o=1).broadcast_to([128, NT, L])
                sloth3 = res["slotbh"][:].rearrange(
                    "p (o l) -> p o l", o=1).broadcast_to([128, NT, L])
                idx3 = idxf[:].rearrange(
                    "p (t o) -> p t o", o=1).broadcast_to([128, NT, L])
                btf3 = btf[:].rearrange("p (t l) -> p t l", l=L)
                nc.vector.tensor_tensor(out=btf3, in0=btf3, in1=slot3, op=SUB)
                c1 = wk.tile([128, NT * L], f32, tag="c1")
                c13 = c1[:].rearrange("p (t l) -> p t l", l=L)
                nc.vector.tensor_tensor(out=c13, in0=idx3, in1=slot3,
                                        op=A_.is_ge)
                c2 = wk.tile([128, NT * L], f32, tag="c2")
                c23 = c2[:].rearrange("p (t l) -> p t l", l=L)
                nc.vector.tensor_tensor(out=c23, in0=idx3, in1=sloth3,
                                        op=A_.is_lt)
                nc.vector.tensor_tensor(out=c1[:], in0=c1[:], in1=c2[:],
                                        op=MULT)
                nc.vector.tensor_tensor(out=btf[:], in0=btf[:], in1=c1[:],
                                        op=MULT)
                gidxf = wk.tile([128, NT], f32, tag="gidxf")
                nc.vector.tensor_reduce(
                    out=gidxf[:], in_=btf[:].rearrange("p (t l) -> p t l", l=L),
                    axis=X, op=ADD)
                nc.vector.tensor_tensor(out=gidxf[:], in0=gidxf[:],
                                        in1=idxf[:], op=ADD)
                gidx = fin.tile([128, NT], i32, tag=f"gidx{d}",
                                name=f"gidx{d}")
                nc.vector.tensor_copy(out=gidx[:], in_=gidxf[:])
                gidx_tiles[d] = gidx
                nc.sync.dma_start(out=d_idx[d][:], in_=gidx[:])
                # issue tail gathers now so they overlap the other
                # direction's decode work
                gath = fin.tile([128, NT * 8], f32, tag=f"gath{d}",
                                name=f"gath{d}")
                for t in range(NT):
                    nc.gpsimd.indirect_dma_start(
                        out=gath[:, t * 8:(t + 1) * 8], out_offset=None,
                        in_=din[f"tab{d}"][:],
                        in_offset=bass.IndirectOffsetOnAxis(
                            ap=gidx[:, t:t + 1], axis=0),
                    )
                gath_tiles[d] = gath

            # ---- tail: exact recompute + normal terms
            sums = fin.tile([128, 4], f32)
            maskt = res["mask"]
            for j, d in enumerate(("x", "y")):
                gath = gath_tiles[d]
                Lc = res[f"loc{d}"][:]
                L3 = Lc.rearrange("p (t k) -> p t k", k=8)
                G3 = gath[:].rearrange("p (t k) -> p t k", k=8)
                prod = wk3.tile([128, NT * 3], f32, tag="prod")
                nc.vector.tensor_tensor(
                    out=prod[:].rearrange("p (t k) -> p t k", k=3),
                    in0=L3[:, :, 0:3], in1=G3[:, :, 0:3], op=MULT)
                dot = wk3.tile([128, NT], f32, tag="sm")
                nc.vector.tensor_reduce(
                    out=dot[:], in_=prod[:].rearrange("p (t k) -> p t k", k=3),
                    axis=X, op=ADD)
                cham = wk3.tile([128, NT], f32, tag="sm")
                nc.vector.tensor_tensor(
                    out=cham[:], in0=L3[:, :, 3], in1=G3[:, :, 3], op=ADD)
                dotm2 = wk3.tile([128, NT], f32, tag="sm")
                nc.vector.tensor_scalar(
                    out=dotm2[:], in0=dot[:], scalar1=-2.0, scalar2=None,
                    op0=MULT)
                nc.vector.tensor_tensor(out=cham[:], in0=cham[:], in1=dotm2[:],
                                        op=ADD)
                nc.vector.tensor_tensor(out=cham[:], in0=cham[:],
                                        in1=maskt[:], op=MULT)
                nc.vector.tensor_reduce(out=sums[:, j:j + 1], in_=cham[:],
                                        axis=X, op=ADD)
                nprod = wk3.tile([128, NT * 3], f32, tag="prod")
                nc.vector.tensor_tensor(
                    out=nprod[:].rearrange("p (t k) -> p t k", k=3),
                    in0=L3[:, :, 4:7], in1=G3[:, :, 4:7], op=MULT)
                ndot = wk3.tile([128, NT], f32, tag="sm")
                nc.vector.tensor_reduce(
                    out=ndot[:], in_=nprod[:].rearrange("p (t k) -> p t k", k=3),
                    axis=X, op=ADD)
                den = wk3.tile([128, NT], f32, tag="sm")
                nc.vector.tensor_tensor(out=den[:], in0=L3[:, :, 7],
                                        in1=G3[:, :, 7], op=MULT)
                nc.vector.tensor_scalar(out=den[:], in0=den[:], scalar1=EPS,
                                        scalar2=None, op0=MAX)
                rec = wk3.tile([128, NT], f32, tag="sm")
                nc.vector.reciprocal(out=rec[:], in_=den[:])
                cos = wk3.tile([128, NT], f32, tag="sm")
                nc.vector.tensor_tensor(out=cos[:], in0=ndot[:], in1=rec[:],
                                        op=MULT)
                acos = wk3.tile([128, NT], f32, tag="sm")
                nc.scalar.activation(out=acos[:], in_=cos[:], func=Abs)
                nterm = wk3.tile([128, NT], f32, tag="sm")
                nc.scalar.activation(out=nterm[:], in_=acos[:], func=Copy,
                                     scale=-1.0, bias=1.0)
                nc.vector.tensor_tensor(out=nterm[:], in0=nterm[:],
                                        in1=maskt[:], op=MULT)
                nc.vector.tensor_reduce(out=sums[:, 2 + j:3 + j], in_=nterm[:],
                                        axis=X, op=ADD)

            # ---- partition reduce + weights + allreduce
            ones = fin.tile([128, 1], f32)
            nc.vector.memset(ones[:], 1.0)
            ps4full = pscan.tile([128, 512], f32, tag="scan", name="ps4full")
            ps4 = ps4full[:4, :1]
            nc.tensor.matmul(out=ps4, lhsT=sums[:], rhs=ones[:],
                             start=True, stop=True)
            sb4 = fin.tile([4, 1], f32)
            nc.scalar.activation(out=sb4[:], in_=ps4, func=Copy)
            nc.sync.dma_start(out=d_sums[:], in_=sb4[:])
            w4 = fin.tile([4, 1], f32)
            nc.sync.dma_start(out=w4[:], in_=din["w4"][:])
            ps1full = pscan.tile([128, 512], f32, tag="scan", name="ps1full")
            ps1 = ps1full[:1, :1]
            nc.tensor.matmul(out=ps1, lhsT=w4[:], rhs=sb4[:],
                             start=True, stop=True)
            sres = fin.tile([1, 1], f32)
            nc.scalar.activation(out=sres[:], in_=ps1, func=Copy)

            cc_in = dram.tile([1, 1], f32)
            cc_out = dram.tile([1, 1], f32)
            nc.sync.dma_start(out=cc_in[:], in_=sres[:])
            nc.gpsimd.collective_compute(
                "AllReduce", ADD,
                replica_groups=[list(range(NCORES))],
                ins=[cc_in.opt()], outs=[cc_out.opt()],
            )
            resid = fin.tile([1, 1], f32)
            nc.sync.dma_start(out=resid[:], in_=cc_out[:])
            nc.sync.dma_start(out=d_out[:], in_=resid[:])

    nc.compile()
    return nc


def _prepare_inputs(points_pred, normals_pred, points_gt, normals_gt, params):
    NT, NCORES, NPAD = params["ntiles"], params["ncores"], params["npad"]
    n, m = params["n"], params["m"]
    pp64 = points_pred.astype(np.float64)
    gg64 = points_gt.astype(np.float64)
    po = morton_order(pp64)
    go = morton_order(gg64)
    pp_s, pn_s = pp64[po], normals_pred.astype(np.float64)[po]
    gg_s, gn_s = gg64[go], normals_gt.astype(np.float64)[go]

    # B-side (candidate) tables per direction
    blkx, centx, radbcx, tabx = _side_tables(gg_s, gn_s, NPAD)   # x scans gt
    blky, centy, radbcy, taby = _side_tables(pp_s, pn_s, NPAD)   # y scans pred

    ibuf = np.broadcast_to(
        (np.int64(0x7F7F0000) | np.arange(CAND, dtype=np.int64))
        .astype(np.int32), (128, CAND)).copy()
    slotb = np.broadcast_to(
        (np.arange(L, dtype=np.float32) * 128), (128, L)).copy()
    slotbh = slotb + 128.0
    w4 = np.array(
        [[CHAMFER_W / n], [CHAMFER_W / m],
         [NORMAL_W / n], [NORMAL_W / m]], dtype=np.float32)

    local = NPAD // NCORES
    ppad = np.full((NPAD, 3), SENTINEL)
    ppad[:n] = pp_s
    gpad = np.full((NPAD, 3), SENTINEL)
    gpad[:m] = gg_s
    pnpad = np.zeros((NPAD, 3))
    pnpad[:n] = pn_s
    gnpad = np.zeros((NPAD, 3))
    gnpad[:m] = gn_s

    in_maps = []
    for i in range(NCORES):
        sel = slice(i * local, (i + 1) * local)
        lhsx = _aug_lhs(ppad[sel]).astype(ml_dtypes.bfloat16)
        lhsy = _aug_lhs(gpad[sel]).astype(ml_dtypes.bfloat16)
        locx = _loc_table(ppad[sel], pnpad[sel], NT)
        locy = _loc_table(gpad[sel], gnpad[sel], NT)
        gi = np.arange(i * local, (i + 1) * local)
        maskv = (gi < n).astype(np.float32)  # n == m
        mask = maskv.reshape(NT, 128).T.copy()
        in_maps.append({
            "lhsx": lhsx, "lhsy": lhsy,
            "blkx": blkx, "blky": blky,
            "centx": centx, "centy": centy,
            "radbcx": radbcx, "radbcy": radbcy,
            "locx": locx, "locy": locy,
            "tabx": tabx, "taby": taby,
            "mask": mask, "w4": w4, "ibuf": ibuf, "slotb": slotb,
            "slotbh": slotbh,
        })
    return in_maps


def _params_for(n, m, ncores=8):
    npad = ((max(n, m) + B * ncores - 1) // (B * ncores)) * (B * ncores)
    return {
        "n": n, "m": m, "ncores": ncores, "npad": npad,
        "ntiles": npad // B // ncores,
        "nblocks": npad // B,
    }


def run(points_pred, normals_pred, points_gt, normals_gt, ncores=8, **runkw):
    b, n, _ = points_pred.shape
    m = points_gt.shape[1]
    assert b == 1
    params = _params_for(n, m, ncores)
    key = (n, m, ncores)
    if key not in _CACHE:
        _CACHE[key] = _build(params)
    nc = _CACHE[key]
    in_maps = _prepare_inputs(
        points_pred[0], normals_pred[0], points_gt[0], normals_gt[0], params)
    from concourse.bass_utils import run_bass_kernel_spmd
    return run_bass_kernel_spmd(nc, in_maps, list(range(ncores)), **runkw)


def kernel(points_pred, normals_pred, points_gt, normals_gt):
    r = run(points_pred, normals_pred, points_gt, normals_gt)
    return np.float32(r.results[0]["out"][0, 0])
